# revision 8
# baseline (speedup 1.0000x reference)
"""Trainium2 Bass kernel for the bbox corner-chamfer loss.

Math: the reference builds 8 corners for each source/target box, forms the
8x8 squared-distance matrix per box pair, takes min over target corners and
means everything.  In the target box's local frame the target corners are the
axis-aligned set {+-hx} x {+-hy} x {+-hz}, so the min over the 8 target
corners separates per coordinate:

    min_j |y_k - q_j|^2 = sum_d (|y_k,d| - |h_d|)^2

where y_k = R_t^T (s_corner_k - c_t).  This removes the 8x8 matrix entirely:
per box we rotate the source box into the target frame (W = R_t^T R_s via
Givens updates, m = R_t^T (c_s - c_t)), build the 8 source corners there, and
accumulate (|y|-H)^2.

Layout: data-parallel over boxes: 8 cores x (128 partitions x 256 lanes).
Every per-box scalar lives in a [128, 256] fp32 SBUF tile; all compute is
elementwise on the Vector/Scalar engines.  Each core emits a [128,1] partial
sum; the host sums and divides by N*8.
"""

import numpy as np

N_TOTAL = 262144
N_CORES = 8
P = 128
NB = N_TOTAL // N_CORES // P  # 256 boxes per partition
PI = float(np.pi)

_CACHE = {}


def _build_nc():
    from concourse import bacc, bass, tile, mybir

    F32 = mybir.dt.float32
    ALU = mybir.AluOpType
    SIN = mybir.ActivationFunctionType.Sin
    SQ = mybir.ActivationFunctionType.Square
    ABSF = mybir.ActivationFunctionType.Abs

    # Bacc (not raw Bass): its compile pipeline splits per-instruction sync
    # waits into event semaphores to satisfy the ISA wait-slot limits and
    # lowers custom-ISA instructions for walrus.
    nc = bacc.Bacc(None)
    src = nc.declare_dram_parameter("source", [P * NB, 9], F32, isOutput=False)
    tgt = nc.declare_dram_parameter("target", [P * NB, 9], F32, isOutput=False)
    outp = nc.declare_dram_parameter("partial", [P, 1], F32, isOutput=True)

    with tile.TileContext(nc) as tc:
        from contextlib import ExitStack

        with ExitStack() as ctx:
            persist = ctx.enter_context(tc.tile_pool(name="persist", bufs=1))
            work = ctx.enter_context(tc.tile_pool(name="work", bufs=14))
            cor = ctx.enter_context(tc.tile_pool(name="cor", bufs=10))

            V, A = nc.vector, nc.scalar

            def T(tag, shape=(P, NB)):
                return persist.tile(list(shape), F32, tag=tag, name=tag)[:]

            def TW(shape=(P, NB)):
                return work.tile(list(shape), F32, tag="w", name="w")[:]

            def TC(shape=(P, NB)):
                return cor.tile(list(shape), F32, tag="c", name="c")[:]

            IN_S = T("in_s", (P, NB, 9))
            IN_T = T("in_t", (P, NB, 9))
            sv = src[:].rearrange("(p b) f -> p b f", p=P)
            tv = tgt[:].rearrange("(p b) f -> p b f", p=P)
            CH = NB // 4
            for q in range(4):
                sl = slice(q * CH, (q + 1) * CH)
                nc.sync.dma_start(out=IN_S[:, sl, :], in_=sv[:, sl, :])
                nc.sync.dma_start(out=IN_T[:, sl, :], in_=tv[:, sl, :])

            # One tiny vector op per DMA chunk: each carries exactly one
            # DMA-semaphore wait, so the ISA per-instruction sync-wait limit is
            # never exceeded.  Later vector ops see the DMA sems as already
            # observed in the engine's vector clock and need no waits at all.
            touch = T("touch", (P, 8))
            for q in range(4):
                i0 = q * CH
                nc.vector.tensor_tensor(
                    touch[:, 2 * q : 2 * q + 1], IN_S[:, i0 : i0 + 1, 0], IN_S[:, i0 : i0 + 1, 0], mybir.AluOpType.bypass
                )
                nc.vector.tensor_tensor(
                    touch[:, 2 * q + 1 : 2 * q + 2], IN_T[:, i0 : i0 + 1, 0], IN_T[:, i0 : i0 + 1, 0], mybir.AluOpType.bypass
                )

            def f(t3, i):
                return t3[:, :, i]

            # --- signed-value helpers (signs tracked at build time) ---
            class SV:
                __slots__ = ("ap", "s")

                def __init__(self, ap, s=1):
                    self.ap, self.s = ap, s

            def MUL(a, b, out=None):
                o = out if out is not None else TW()
                V.tensor_mul(o, a.ap, b.ap)
                return SV(o, a.s * b.s)

            def ADD(a, b, out=None):  # a + b with symbolic signs
                o = out if out is not None else TW()
                if a.s == b.s:
                    V.tensor_add(o, a.ap, b.ap)
                    return SV(o, a.s)
                if a.s > 0:
                    V.tensor_sub(o, a.ap, b.ap)
                else:
                    V.tensor_sub(o, b.ap, a.ap)
                return SV(o, 1)

            def SUB(a, b, out=None):
                return ADD(a, SV(b.ap, -b.s), out)

            # --- trig: sin/cos of (sa0-ta0, sa1, sa2, ta0, ta1, ta2) ---
            d0 = T("d0")
            V.tensor_sub(d0, f(IN_S, 6), f(IN_T, 6))
            angle_aps = [d0, f(IN_S, 7), f(IN_S, 8), f(IN_T, 6), f(IN_T, 7), f(IN_T, 8)]
            trig = []
            for i, ap in enumerate(angle_aps):
                ws = TW()
                V.add_range_wrap(ws, ap, 0.0, PI, 2 * PI)
                wc = TW()
                V.add_range_wrap(wc, ap, PI / 2, PI, 2 * PI)
                sn = T(f"s{i}")
                A.activation(sn, ws, SIN)
                cn = T(f"c{i}")
                A.activation(cn, wc, SIN)
                trig.append((SV(sn), SV(cn)))
            (sd0, cd0), (ss1, cs1), (ss2, cs2), (st0, ct0), (st1, ct1), (st2, ct2) = trig

            # --- R = Rz(d0) Rx(sa1) Ry(sa2) ---
            x12 = MUL(ss1, ss2, T("x12"))
            x1c2 = MUL(ss1, cs2, T("x1c2"))
            R00 = SUB(MUL(cd0, cs2), MUL(sd0, x12), T("R00"))
            R02 = ADD(MUL(cd0, ss2), MUL(sd0, x1c2), T("R02"))
            R10 = ADD(MUL(sd0, cs2), MUL(cd0, x12), T("R10"))
            R12 = SUB(MUL(sd0, ss2), MUL(cd0, x1c2), T("R12"))
            R01 = SV(MUL(sd0, cs1, T("n01")).ap, -1)
            R11 = MUL(cd0, cs1, T("R11"))
            R20 = SV(MUL(cs1, ss2, T("n20")).ap, -1)
            R21 = ss1
            R22 = MUL(cs1, cs2, T("R22"))
            R = [[R00, R01, R02], [R10, R11, R12], [R20, R21, R22]]

            # --- A = Rx(-ta1) @ R ; W = Ry(-ta2) @ A  (W = R_t^T R_s) ---
            A1 = [ADD(MUL(ct1, R[1][j]), MUL(st1, R[2][j]), T(f"A1{j}")) for j in range(3)]
            A2 = [SUB(MUL(ct1, R[2][j]), MUL(st1, R[1][j]), T(f"A2{j}")) for j in range(3)]
            W0 = [SUB(MUL(ct2, R[0][j]), MUL(st2, A2[j]), T(f"W0{j}")) for j in range(3)]
            W2 = [ADD(MUL(st2, R[0][j]), MUL(ct2, A2[j]), T(f"W2{j}")) for j in range(3)]
            W = [W0, A1, W2]

            # --- m = Ry(-ta2) Rx(-ta1) Rz(-ta0) (c_s - c_t) ---
            G = []
            for d in range(3):
                g = T(f"G{d}")
                V.tensor_sub(g, f(IN_S, d), f(IN_T, d))
                G.append(SV(g))
            g0 = ADD(MUL(ct0, G[0]), MUL(st0, G[1]), T("g0"))
            g1 = SUB(MUL(ct0, G[1]), MUL(st0, G[0]), T("g1"))
            g2 = G[2]
            m1 = ADD(MUL(ct1, g1), MUL(st1, g2), T("m1"))
            m2a = SUB(MUL(ct1, g2), MUL(st1, g1), T("m2a"))
            m0 = SUB(MUL(ct2, g0), MUL(st2, m2a), T("m0"))
            m2 = ADD(MUL(st2, g0), MUL(ct2, m2a), T("m2"))
            m = [m0, m1, m2]

            # --- half-size-scaled columns of W: u,v,w ; H = |ht/2| ---
            base = []
            for ci in range(3):
                col = []
                for d in range(3):
                    o = T(f"b{ci}{d}")
                    V.scalar_tensor_tensor(o, f(IN_S, 3 + ci), 0.5, W[d][ci].ap, ALU.mult, ALU.mult)
                    col.append(SV(o, W[d][ci].s))
                base.append(col)
            U, Vv, Wv = base

            H = []
            for d in range(3):
                h = T(f"H{d}")
                A.activation(h, f(IN_T, 3 + d), ABSF, scale=0.5)
                H.append(h)

            # --- 8 corners per dim; accumulate (|y| - H)^2 ---
            acc = T("acc", (P, 24))
            sq = [T("sq0"), T("sq1")]
            idx = 0
            for d in range(3):
                ep = ADD(m[d], U[d], TC())
                em = SUB(m[d], U[d], TC())
                for e in (ep, em):
                    fp = ADD(e, Vv[d], TC())
                    fm = SUB(e, Vv[d], TC())
                    for ff in (fp, fm):
                        for sw in (1, -1):
                            y = ADD(ff, SV(Wv[d].ap, Wv[d].s * sw), TC())
                            ay = TC()
                            A.activation(ay, y.ap, ABSF)
                            r = TC()
                            V.tensor_sub(r, ay, H[d])
                            A.activation(sq[idx % 2], r, SQ, accum_out=acc[:, idx : idx + 1])
                            idx += 1

            part = T("part", (P, 1))
            V.tensor_reduce(part, acc, mybir.AxisListType.X, ALU.add)
            nc.sync.dma_start(out=outp[:], in_=part)
    nc.finalize()
    return nc


def _get_nc():
    if "nc" not in _CACHE:
        _CACHE["nc"] = _build_nc()
    return _CACHE["nc"]


def _run(in_maps, **kwargs):
    from concourse.bass_utils import run_bass_kernel_spmd

    return run_bass_kernel_spmd(_get_nc(), in_maps, list(range(N_CORES)), **kwargs)


def _make_in_maps(source, target):
    src = np.ascontiguousarray(np.asarray(source, np.float32)).reshape(N_CORES, P * NB, 9)
    tgt = np.ascontiguousarray(np.asarray(target, np.float32)).reshape(N_CORES, P * NB, 9)
    return [{"source": src[c], "target": tgt[c]} for c in range(N_CORES)]


def _loss_from_results(results):
    tot = 0.0
    for r in results:
        tot += float(r["partial"].astype(np.float64).sum())
    return np.float32(tot / (N_TOTAL * 8))


def kernel(source, target):
    res = _run(_make_in_maps(source, target))
    return _loss_from_results(res.results)


# revision 9
# speedup vs baseline: 1.0932x; 1.0932x over previous
"""Trainium2 Bass kernel for the bbox corner-chamfer loss.

Math: the reference builds the 8 corners of each source/target box, forms
the per-box 8x8 squared-distance matrix, takes the min over target corners
and means everything.  In the target box's local frame the target corners
are the axis-aligned set {+-hx} x {+-hy} x {+-hz}, so the min over the 8
target corners separates per coordinate and the 8x8 matrix disappears:

    loss = mean_{n,k} sum_d (|y_k,d| - |h_t,d|/2)^2,   y_k = R_t^T(S_k - c_t)

Mapping (data-parallel over boxes: 8 cores x 128 partitions x 256 lanes,
one [128,1] fp32 partial sum per core, host sums and divides by 8N):
  - plane-major packing [P, K, NB]: every per-angle / per-dim scalar is a
    dense [P, NB] plane (innermost stride 1), so ops qualify for the DVE
    2x_1P perf mode
  - bf16 compute for the rotation / corner stages (fp32 trig inputs and
    fp32 accumulation), doubling DVE throughput; the loss is a mean of 2M
    terms so the bf16 rounding bias is ~1e-4 relative
"""

import numpy as np

N_TOTAL = 262144
N_CORES = 8
P = 128
NB = N_TOTAL // N_CORES // P  # 256 boxes per partition
PI = float(np.pi)

_CACHE = {}


def _build_nc():
    from contextlib import ExitStack

    from concourse import bacc, tile, mybir

    F32 = mybir.dt.float32
    BF16 = mybir.dt.bfloat16
    ALU = mybir.AluOpType
    SIN = mybir.ActivationFunctionType.Sin
    SQ = mybir.ActivationFunctionType.Square
    ABSF = mybir.ActivationFunctionType.Abs

    nc = bacc.Bacc(None)
    src = nc.declare_dram_parameter("source", [P * NB, 9], F32, isOutput=False)
    tgt = nc.declare_dram_parameter("target", [P * NB, 9], F32, isOutput=False)
    outp = nc.declare_dram_parameter("partial", [P, 1], F32, isOutput=True)

    with tile.TileContext(nc) as tc:
        with ExitStack() as ctx:
            persist = ctx.enter_context(tc.tile_pool(name="persist", bufs=1))
            work1 = ctx.enter_context(tc.tile_pool(name="work1", bufs=8))
            tailp = ctx.enter_context(tc.tile_pool(name="tailp", bufs=2))

            V, A = nc.vector, nc.scalar

            def T(tag, shape=(P, NB), dt=BF16):
                return persist.tile(list(shape), dt, tag=tag, name=tag)[:]

            def TW(shape=(P, NB)):
                return work1.tile(list(shape), BF16, tag="w1", name="w1")[:]

            IN_S = T("in_s", (P, NB, 9), F32)
            IN_T = T("in_t", (P, NB, 9), F32)
            sv = src[:].rearrange("(p b) f -> p b f", p=P)
            tv = tgt[:].rearrange("(p b) f -> p b f", p=P)
            CH = NB // 4
            for q in range(4):
                sl = slice(q * CH, (q + 1) * CH)
                nc.sync.dma_start(out=IN_S[:, sl, :], in_=sv[:, sl, :])
                nc.sync.dma_start(out=IN_T[:, sl, :], in_=tv[:, sl, :])

            # one tiny op per DMA chunk so no later instruction needs more
            # than one DMA-semaphore wait (ISA wait-slot limits)
            touch = T("touch", (P, 8), F32)
            for q in range(4):
                i0 = q * CH
                V.tensor_tensor(touch[:, 2 * q : 2 * q + 1], IN_S[:, i0 : i0 + 1, 0], IN_S[:, i0 : i0 + 1, 0], ALU.bypass)
                V.tensor_tensor(touch[:, 2 * q + 1 : 2 * q + 2], IN_T[:, i0 : i0 + 1, 0], IN_T[:, i0 : i0 + 1, 0], ALU.bypass)

            # --- trig: sin/cos of (sa0-ta0, sa1, sa2, ta0, ta1, ta2) ---
            # per-angle wraps (strided field in, dense plane out, fp32),
            # then one Sin per 6-angle pack; ACT writes bf16 planes
            d0 = T("d0", (P, NB), F32)
            V.tensor_sub(d0, IN_S[:, :, 6], IN_T[:, :, 6])
            SINP = persist.tile([P, 6, NB], F32, tag="trigin", name="sinp")[:]
            COSP = persist.tile([P, 6, NB], F32, tag="trigin", name="cosp")[:]
            sa12 = IN_S[:, :, 7:9].transpose([0, 2, 1])  # [P, 2, NB]
            ta012 = IN_T[:, :, 6:9].transpose([0, 2, 1])  # [P, 3, NB]
            for dst, shift in ((SINP, 0.0), (COSP, PI / 2)):
                V.add_range_wrap(dst[:, 0, :], d0, shift, PI, 2 * PI)
                V.add_range_wrap(dst[:, 1:3, :], sa12, shift, PI, 2 * PI)
                V.add_range_wrap(dst[:, 3:6, :], ta012, shift, PI, 2 * PI)
            S6 = T("s6", (P, 6, NB))
            C6 = T("c6", (P, 6, NB))
            A.activation(S6, SINP, SIN)
            A.activation(C6, COSP, SIN)

            sd0, ss1, ss2, st0, st1, st2 = (S6[:, i, :] for i in range(6))
            cd0, cs1, cs2, ct0, ct1, ct2 = (C6[:, i, :] for i in range(6))

            # --- R = Rz(d0) Rx(sa1) Ry(sa2): rows [P, 3, NB], true signs ---
            R0 = T("R0", (P, 3, NB))
            R1 = T("R1", (P, 3, NB))
            R2 = T("R2", (P, 3, NB))
            x12 = TW()
            V.tensor_mul(x12, ss1, ss2)
            x1c2 = TW()
            V.tensor_mul(x1c2, ss1, cs2)

            def combine(dst, a0, a1, b0, b1, op):
                # dst = a0*a1 (op) b0*b1
                p0, p1 = TW(), TW()
                V.tensor_mul(p0, a0, a1)
                V.tensor_mul(p1, b0, b1)
                V.tensor_tensor(dst, p0, p1, op)

            combine(R0[:, 0, :], cd0, cs2, sd0, x12, ALU.subtract)
            V.scalar_tensor_tensor(R0[:, 1, :], sd0, -1.0, cs1, ALU.mult, ALU.mult)
            combine(R0[:, 2, :], cd0, ss2, sd0, x1c2, ALU.add)
            combine(R1[:, 0, :], sd0, cs2, cd0, x12, ALU.add)
            V.tensor_mul(R1[:, 1, :], cd0, cs1)
            combine(R1[:, 2, :], sd0, ss2, cd0, x1c2, ALU.subtract)
            V.scalar_tensor_tensor(R2[:, 0, :], cs1, -1.0, ss2, ALU.mult, ALU.mult)
            V.tensor_copy(R2[:, 1, :], ss1)
            V.tensor_mul(R2[:, 2, :], cs1, cs2)

            # --- W = Ry(-ta2) Rx(-ta1) R  (rows [P,3,NB], broadcast trig) ---
            def b3(ap1):  # [P,NB] -> [P,3,NB]: stride-0 middle, dense inner
                return ap1.unsqueeze(1).broadcast_to([P, 3, NB])

            def row_combine(dst, c, ra, s, rb, op):
                p0, p1 = TW((P, 3, NB)), TW((P, 3, NB))
                V.tensor_mul(p0, ra, b3(c))
                V.tensor_mul(p1, rb, b3(s))
                V.tensor_tensor(dst, p0, p1, op)

            A1 = T("A1", (P, 3, NB))
            A2 = T("A2", (P, 3, NB))
            row_combine(A1, ct1, R1, st1, R2, ALU.add)
            row_combine(A2, ct1, R2, st1, R1, ALU.subtract)
            W0 = T("W0", (P, 3, NB))
            W2 = T("W2", (P, 3, NB))
            row_combine(W0, ct2, R0, st2, A2, ALU.subtract)
            row_combine(W2, st2, R0, ct2, A2, ALU.add)
            Wrows = [W0, A1, W2]

            # --- m = Ry(-ta2) Rx(-ta1) Rz(-ta0) (c_s - c_t): [P, 3, NB] ---
            GV = T("GV", (P, 3, NB))
            for dd in range(3):
                V.tensor_tensor(GV[:, dd, :], IN_S[:, :, dd], IN_T[:, :, dd], ALU.subtract)
            M3 = T("M3", (P, 3, NB))

            def vec_combine(dst, c, ga, s, gb, op):
                p0, p1 = TW(), TW()
                V.tensor_mul(p0, c, ga)
                V.tensor_mul(p1, s, gb)
                V.tensor_tensor(dst, p0, p1, op)

            g0 = T("g0")
            g1 = T("g1")
            vec_combine(g0, ct0, GV[:, 0, :], st0, GV[:, 1, :], ALU.add)
            vec_combine(g1, ct0, GV[:, 1, :], st0, GV[:, 0, :], ALU.subtract)
            m2a = T("m2a")
            vec_combine(M3[:, 1, :], ct1, g1, st1, GV[:, 2, :], ALU.add)
            vec_combine(m2a, ct1, GV[:, 2, :], st1, g1, ALU.subtract)
            vec_combine(M3[:, 0, :], ct2, g0, st2, m2a, ALU.subtract)
            vec_combine(M3[:, 2, :], st2, g0, ct2, m2a, ALU.add)

            # --- scaled columns: U/Vv/Wv [P,3,NB]; H [P,3,NB] = |ht/2| ---
            hs = [T(f"hs{c}", (P, NB)) for c in range(3)]
            for c in range(3):
                V.tensor_scalar(hs[c], IN_S[:, :, 3 + c], 0.5, None, ALU.mult)
            U = T("U", (P, 3, NB))
            Vv = T("Vv", (P, 3, NB))
            Wv = T("Wv", (P, 3, NB))
            for dd in range(3):
                V.tensor_mul(U[:, dd, :], hs[0], Wrows[dd][:, 0, :])
                V.tensor_mul(Vv[:, dd, :], hs[1], Wrows[dd][:, 1, :])
                V.tensor_mul(Wv[:, dd, :], hs[2], Wrows[dd][:, 2, :])
            H3 = T("H3", (P, 3, NB))
            for dd in range(3):
                A.activation(H3[:, dd, :], IN_T[:, :, 3 + dd], ABSF, scale=0.5)

            # --- corners: EE [P,2,3,NB], FF [P,4,3,NB] ---
            EE = T("EE", (P, 2, 3, NB))
            V.tensor_add(EE[:, 0, :, :], M3, U)
            V.tensor_sub(EE[:, 1, :, :], M3, U)
            FF = T("FF", (P, 4, 3, NB))
            vb = Vv.unsqueeze(1).broadcast_to([P, 2, 3, NB])
            V.tensor_add(FF[:, 0:2, :, :], EE, vb)
            V.tensor_sub(FF[:, 2:4, :, :], EE, vb)

            # --- tail: per f-slice corner pair, (|y| - H)^2 accumulated ---
            acc = T("acc", (P, 4), F32)
            hb = H3.unsqueeze(1).broadcast_to([P, 2, 3, NB])
            wb = Wv.unsqueeze(1)

            def TT(tag):
                return tailp.tile([P, 2, 3, NB], BF16, tag=tag, name=tag)[:]

            for g in range(4):
                ff = FF[:, g, :, :].unsqueeze(1)
                yp = TT("yp")
                V.tensor_add(yp[:, 0:1, :, :], ff, wb)
                V.tensor_sub(yp[:, 1:2, :, :], ff, wb)
                ay = TT("ay")
                A.activation(ay, yp, ABSF)
                rr = TT("rr")
                V.tensor_sub(rr, ay, hb)
                sqo = TT("sqo")
                A.activation(sqo, rr, SQ, accum_out=acc[:, g : g + 1])

            part = T("part", (P, 1), F32)
            V.tensor_reduce(part, acc, mybir.AxisListType.X, ALU.add)
            nc.sync.dma_start(out=outp[:], in_=part)
    nc.finalize()
    return nc


def _get_nc():
    if "nc" not in _CACHE:
        _CACHE["nc"] = _build_nc()
    return _CACHE["nc"]


def _run(in_maps, **kwargs):
    from concourse.bass_utils import run_bass_kernel_spmd

    return run_bass_kernel_spmd(_get_nc(), in_maps, list(range(N_CORES)), **kwargs)


def _make_in_maps(source, target):
    src = np.ascontiguousarray(np.asarray(source, np.float32)).reshape(N_CORES, P * NB, 9)
    tgt = np.ascontiguousarray(np.asarray(target, np.float32)).reshape(N_CORES, P * NB, 9)
    return [{"source": src[c], "target": tgt[c]} for c in range(N_CORES)]


def _loss_from_results(results):
    tot = 0.0
    for r in results:
        tot += float(r["partial"].astype(np.float64).sum())
    return np.float32(tot / (N_TOTAL * 8))


def kernel(source, target):
    res = _run(_make_in_maps(source, target))
    return _loss_from_results(res.results)
